# revision 1
# baseline (speedup 1.0000x reference)
"""Trainium2 Bass kernel for nn_NodeEncoder (per-type Linear over interleaved node types).

Problem: x [800000, 128] f32, W [8, 256, 128], b [8, 256].
Node n has type k = n % 8; y[n] = (W[k] * mask_k) @ x[n] + b[k], y [800000, 256].

Strategy (8 cores, data-parallel over graphs, weights replicated):
  - Each core gets 100000 consecutive nodes (12500 graphs), padded to
    100352 = 49 super-tiles of 2048 nodes (256 graphs).
  - x is cast to fp16 (round-to-nearest; the PE multiplies fp16 at FP22 so
    ~2.4e-4 per-element rel err) and laid out on the host in transposed
    slice form: x_in[s, d, 128*j + n] = x[2048*s + 16*n + j, d].  Each
    slice j of a super-tile is 128 nodes, ALL of type j%8, with the
    contraction dim d already on partitions — so a contiguous 512 KiB DMA
    per super-tile feeds matmuls directly, no on-device transpose.
  - For types with dim < 128 the host writes 1.0 into x column `dim`
    (masked region), so rows 0..dim of a slice are [x.T; ones] and the
    bias rides as contraction row `dim` of the weight tile
    (y = [x,1] @ [W^T; b]).  For the two dim-128 types the (exact fp32)
    bias is added by GpSimd after eviction.
  - fp16 matmul accumulates in fp32 PSUM; pairs of slices share one PSUM
    bank [128, 512] and ScalarE/VectorE alternate evicting two slices per
    op into the fp32 out tile [128, 4096], which maps linearly to 2048
    output rows -> one contiguous 2 MiB DMA out.  All DMAs are fully
    contiguous.
W is pre-masked + pre-transposed on host (it is tiny: 1 MB).
"""

import os
import sys

import numpy as np

for _p in ("/root/.axon_site", "/root/.axon_site/_ro/trn_rl_repo", "/root/.axon_site/_ro/pypackages"):
    if os.path.isdir(_p) and _p not in sys.path:
        sys.path.append(_p)

import concourse.bass as bass
import concourse.mybir as mybir
import concourse.tile as tile
from concourse import bacc
from concourse.bass_utils import run_bass_kernel_spmd

N_TYPES = 8
MAX_DIM = 128
FEAT = 256
N_GRAPHS = 100000
NODE_DIMS = np.array([16, 32, 64, 128, 64, 32, 16, 128], dtype=np.int32)

N_CORES = 8
NODES_PER_CORE = N_GRAPHS * N_TYPES // N_CORES  # 100000
SUPER_NODES = 2048          # nodes per super-tile (256 graphs)
N_SUPER = 49                # super-tiles per core
PAD_NODES = SUPER_NODES * N_SUPER  # 100352
SLICES = SUPER_NODES // 128  # 16 slices of 128 nodes per super-tile
UNIT = 7                    # super-tiles per DMA unit (49 = 7 units of 7)
N_UNITS = N_SUPER // UNIT

_F32 = mybir.dt.float32
_F16 = mybir.dt.float16
OUT_F16 = True              # store y as fp16 (halves write traffic; host upcasts)

# PE row-strip packing: each type's contraction rows live at STRIP[k] so pairs
# of matmuls with disjoint row-groups run concurrently in the PE array:
#   (t2@0, t4@64) 64+64, (t1@0, t5@64) 33 rounds to 64, (t0@0, t6@32) 17->32,
#   t3 and t7 use the full 128 rows.
# KK[k] = contraction rows; types 0,1,5,6 append a ones-row (bias folded into
# the weight tile); types 2,4 have dim 64 (65 would round to a full-array
# tile) and types 3,7 have dim 128 — their bias is added during eviction.
STRIP = {0: 0, 1: 0, 2: 0, 3: 0, 4: 64, 5: 64, 6: 32, 7: 0}
KK = {0: 17, 1: 33, 2: 64, 3: 128, 4: 64, 5: 33, 6: 17, 7: 128}
MM_ORDER = [2, 4, 1, 5, 0, 6, 3, 7]  # pack members adjacent on the PE queue
# x ships dense: only the KK[k] real contraction rows per type (484 of 1024
# rows per slice-group); the per-type DMA scatters them to the strip rows.
R_OFF = {}
_r = 0
for _k in range(N_TYPES):
    R_OFF[_k] = _r
    _r += KK[_k]
DENSE_ROWS = _r  # 484
_nc_cache = {}


def _build_nc():
    if "nc" in _nc_cache:
        return _nc_cache["nc"]
    out_dt = _F16 if OUT_F16 else _F32
    nc = bacc.Bacc("TRN2", target_bir_lowering=False, debug=False)
    x = nc.dram_tensor("x", [N_UNITS, DENSE_ROWS, UNIT * 2 * 128], _F16, kind="ExternalInput").ap()
    wtb = nc.dram_tensor("wtb", [128, N_TYPES * FEAT], _F16, kind="ExternalInput").ap()
    # bias tiles for the unfolded-bias types, broadcast over partitions:
    # [0:512] = [b2|b3] (pair eviction), [512:768] = b4, [768:1024] = b7
    bias_pair = nc.dram_tensor("bias_pair", [128, 4 * FEAT], _F32, kind="ExternalInput").ap()
    y = nc.dram_tensor("y", [N_UNITS, 128, UNIT * SLICES * FEAT], out_dt, kind="ExternalOutput").ap()

    with tile.TileContext(nc) as tc:
        with (
            tc.tile_pool(name="const", bufs=1) as const,
            tc.tile_pool(name="xin", bufs=2) as xin_pool,
            tc.tile_pool(name="outsb", bufs=2) as out_pool,
            tc.tile_pool(name="ps_o", bufs=6, space="PSUM") as ps_o,
        ):
            wtb_sb = const.tile([128, N_TYPES * FEAT], _F16)
            nc.sync.dma_start(wtb_sb[:], wtb[:])
            bp_sb = const.tile([128, 4 * FEAT], _F32)
            nc.sync.dma_start(bp_sb[:], bias_pair[:])

            for u in range(N_UNITS):
                xs = xin_pool.tile([128, UNIT * SUPER_NODES], _F16)
                xs4 = xs[:].rearrange(
                    "p (s t n) -> p s t n", s=UNIT, t=SLICES, n=128
                )
                for k in range(N_TYPES):
                    kk, sp = KK[k], STRIP[k]
                    nc.sync.dma_start(
                        xs4[sp:sp + kk, :, k::N_TYPES, :],
                        x[u, R_OFF[k]:R_OFF[k] + kk, :].rearrange(
                            "p (s t n) -> p s t n", s=UNIT, t=2, n=128
                        ),
                    )
                out_sb = out_pool.tile([128, UNIT * SLICES * FEAT], out_dt)
                for st in range(UNIT):
                    xoff = st * SUPER_NODES
                    ooff = st * SLICES * FEAT
                    for g in range(2):  # two 8-slice type-groups per super-tile
                        pos = [
                            ps_o.tile([128, 2 * FEAT], _F32, tag="po", name=f"po_{u}_{st}_{g}_{i}")
                            for i in range(4)
                        ]
                        for kt in MM_ORDER:
                            j = g * N_TYPES + kt
                            kk, sp = KK[kt], STRIP[kt]
                            nc.tensor.matmul(
                                pos[kt // 2][:, (kt % 2) * FEAT:(kt % 2 + 1) * FEAT],
                                xs[sp:sp + kk, xoff + j * 128:xoff + (j + 1) * 128],
                                wtb_sb[sp:sp + kk, kt * FEAT:(kt + 1) * FEAT],
                                start=True, stop=True,
                            )
                        # evictions: biased halves on DVE tensor_add (bias folded
                        # into the PSUM->SBUF move, single fp16 rounding),
                        # unbiased halves on ScalarE copy.
                        jb = g * N_TYPES
                        oss = [
                            out_sb[:, ooff + (jb + i) * FEAT:ooff + (jb + i + 1) * FEAT]
                            for i in range(N_TYPES)
                        ]
                        nc.scalar.copy(out_sb[:, ooff + jb * FEAT:ooff + (jb + 2) * FEAT], pos[0][:])
                        nc.vector.tensor_add(
                            out_sb[:, ooff + (jb + 2) * FEAT:ooff + (jb + 4) * FEAT],
                            pos[1][:], bp_sb[:, 0:2 * FEAT],
                        )
                        nc.vector.tensor_add(oss[4], pos[2][:, 0:FEAT], bp_sb[:, 2 * FEAT:3 * FEAT])
                        nc.scalar.copy(oss[5], pos[2][:, FEAT:2 * FEAT])
                        nc.scalar.copy(oss[6], pos[3][:, 0:FEAT])
                        nc.vector.tensor_add(oss[7], pos[3][:, FEAT:2 * FEAT], bp_sb[:, 3 * FEAT:4 * FEAT])
                # split the final store so the tail drains incrementally
                if u == N_UNITS - 1:
                    for st in range(UNIT):
                        nc.scalar.dma_start(
                            y[u][:, st * SLICES * FEAT:(st + 1) * SLICES * FEAT],
                            out_sb[:, st * SLICES * FEAT:(st + 1) * SLICES * FEAT],
                        )
                else:
                    nc.scalar.dma_start(y[u], out_sb[:])

    nc.finalize()
    _nc_cache["nc"] = nc
    return nc


def _prep_weights(W, b):
    mask = (np.arange(MAX_DIM)[None, None, :] < NODE_DIMS[:, None, None])
    W_eff = np.where(mask, W, 0).astype(np.float32)  # [T, F, D]
    # wtb[:, k*256+f]: W_eff[k].T at rows STRIP[k]..STRIP[k]+dim_k, then (for
    # types with a folded bias) b[k] at row STRIP[k]+dim_k.
    wtb = np.zeros((MAX_DIM, N_TYPES * FEAT), dtype=np.float32)
    for k in range(N_TYPES):
        dim, sp, kk = int(NODE_DIMS[k]), STRIP[k], KK[k]
        wtb[sp:sp + dim, k * FEAT:(k + 1) * FEAT] = W_eff[k, :, :dim].T
        if kk == dim + 1:
            wtb[sp + dim, k * FEAT:(k + 1) * FEAT] = b[k]
    # bias_pair [128, 1024] f32: [b2 | b3 | b4 | b7] broadcast over partitions
    bp = np.concatenate([b[2], b[3], b[4], b[7]]).astype(np.float32)[None, :]
    bias_pair = np.ascontiguousarray(np.broadcast_to(bp, (128, 4 * FEAT)))
    return wtb.astype(np.float16), bias_pair


def _prep_x_shard(x, c):
    """fp16, ones-column injected, dense transposed per-type layout:
    xd[u, R_OFF[k] + d, ((st*2 + jj)*128 + n)] = xc[2048*(7u+st) + 16*n + (k+8*jj), d]
    for d < KK[k] (the device DMA scatters rows to partition STRIP[k]+d)."""
    xc = np.zeros((PAD_NODES, MAX_DIM), dtype=np.float32)
    xc[:NODES_PER_CORE] = x[c * NODES_PER_CORE:(c + 1) * NODES_PER_CORE]
    for k in range(N_TYPES):
        dim = int(NODE_DIMS[k])
        if KK[k] == dim + 1:
            xc[k::N_TYPES, dim] = 1.0  # ones-row for the folded bias
    xh = xc.astype(np.float16).reshape(N_SUPER, 128, SLICES, MAX_DIM)  # [s, n, j, d]
    xt = np.ascontiguousarray(xh.transpose(0, 3, 2, 1))  # [s, d, j, n]
    xr = xt.reshape(N_UNITS, UNIT, MAX_DIM, SLICES, 128)  # [u, st, d, j, n]
    xd = np.empty((N_UNITS, DENSE_ROWS, UNIT * 2 * 128), dtype=np.float16)
    for k in range(N_TYPES):
        kk = KK[k]
        blk = xr[:, :, :kk, k::N_TYPES, :]          # [u, st, kk, 2, n]
        blk = blk.transpose(0, 2, 1, 3, 4)          # [u, kk, st, 2, n]
        xd[:, R_OFF[k]:R_OFF[k] + kk, :] = blk.reshape(N_UNITS, kk, UNIT * 2 * 128)
    return xd


def run(x, W, b, trace=False):
    nc = _build_nc()
    wtb, bias_pair = _prep_weights(W, b)
    in_maps = []
    for c in range(N_CORES):
        in_maps.append({
            "x": _prep_x_shard(x, c),
            "wtb": wtb,
            "bias_pair": bias_pair,
        })
    res = run_bass_kernel_spmd(nc, in_maps, list(range(N_CORES)), trace=trace)
    y = np.empty((N_GRAPHS * N_TYPES, FEAT), dtype=np.float32)
    for c in range(N_CORES):
        yu = np.asarray(res.results[c]["y"]).reshape(N_UNITS, 128, UNIT, SLICES * FEAT)
        yc = yu.transpose(0, 2, 1, 3).reshape(PAD_NODES, FEAT)
        y[c * NODES_PER_CORE:(c + 1) * NODES_PER_CORE] = yc[:NODES_PER_CORE].astype(np.float32)
    return y, res


def kernel(**inputs):
    y, _ = run(inputs["x"], inputs["W"], inputs["b"])
    return y


if __name__ == "__main__":
    rng = np.random.default_rng(0)
    x = rng.standard_normal((N_GRAPHS * N_TYPES, MAX_DIM), dtype=np.float32)
    W = (rng.standard_normal((N_TYPES, FEAT, MAX_DIM), dtype=np.float32) * 0.05)
    b = (rng.standard_normal((N_TYPES, FEAT), dtype=np.float32) * 0.05)
    y, res = run(x, W, b)
    mask = (np.arange(MAX_DIM)[None, None, :] < NODE_DIMS[:, None, None])
    W_eff = np.where(mask, W, 0).astype(np.float32)
    idx = rng.integers(0, N_GRAPHS * N_TYPES, 256)
    exp = np.stack([W_eff[n % 8] @ x[n] + b[n % 8] for n in idx])
    act = y[idx]
    err = np.abs(act - exp).max() / (np.abs(exp).max() + 1e-30)
    print("spot-check rel err:", err)



# revision 3
# speedup vs baseline: 1.3436x; 1.3436x over previous
"""Trainium2 Bass kernel for nn_NodeEncoder (per-type Linear over interleaved node types).

Problem: x [800000, 128] f32, W [8, 256, 128], b [8, 256].
Node n has type k = n % 8; y[n] = (W[k] * mask_k) @ x[n] + b[k], y [800000, 256].

Strategy (8 cores, data-parallel over graphs, weights replicated):
  - Each core gets 100000 consecutive nodes (12500 per type), padded to
    12544 = 28*448 nodes per type.
  - Weight-stationary matmuls: lhsT = W[k] half [kk, 128 feats] (tiny),
    rhs = x streamed [kk, 512 nodes] per chunk, out = PSUM [128 feats,
    512 nodes] fp32 (one full 2 KiB bank).  25 chunks (24x512 + 256)
    per (type, feat-half); 400 matmuls per core total.
  - x ships fp16, type-major: xd[R_OFF[k] + d, i] = x_typek[i, d], each
    type's block fully contiguous -> 8 large input DMAs per core with
    maximal descriptors (25 KiB per partition row).
  - For types with dim < 128 a ones-row is appended (bias rides as an
    extra contraction row of the weight tile).  Types 3,7 (dim 128) get
    their bias during PSUM eviction: ACT activation-bias / DVE
    tensor_scalar_add with a per-partition [128,1] bias vector.
  - Evictions (PSUM fp32 -> SBUF fp16) alternate Scalar:Vector 5:4
    (matching their 1.2 / 0.96 GHz rates).  Output y is written
    feat-major [16 blocks (k,h), 128 feats, 12544 nodes] fp16, each
    (k,h) block one contiguous 3.2 MiB DMA issued from the (otherwise
    idle) GpSimd queue; the host untangles to node-major fp32.
"""

import os
import sys

import numpy as np

for _p in ("/root/.axon_site", "/root/.axon_site/_ro/trn_rl_repo", "/root/.axon_site/_ro/pypackages"):
    if os.path.isdir(_p) and _p not in sys.path:
        sys.path.append(_p)

import concourse.bass as bass
import concourse.mybir as mybir
import concourse.tile as tile
from concourse import bacc
from concourse.bass_utils import run_bass_kernel_spmd

N_TYPES = 8
MAX_DIM = 128
FEAT = 256
N_GRAPHS = 100000
NODE_DIMS = np.array([16, 32, 64, 128, 64, 32, 16, 128], dtype=np.int32)

N_CORES = 8
NODES_PER_CORE = N_GRAPHS * N_TYPES // N_CORES  # 100000
NPT_REAL = NODES_PER_CORE // N_TYPES            # 12500 nodes per type per core
NPT = 12544                                     # padded: 28 * 448 = 24.5 * 512
CHUNKS = [512] * 24 + [256]                     # sum = 12544

_F32 = mybir.dt.float32
_F16 = mybir.dt.float16

# kk = contraction rows per type: dim + 1 (ones-row folds the bias) for
# dim < 128; types 3,7 use all 128 rows and get bias at eviction.
FOLD = [int(d) < MAX_DIM for d in NODE_DIMS]
KK = [int(d) + (1 if f else 0) for d, f in zip(NODE_DIMS, FOLD)]
R_OFF = np.concatenate([[0], np.cumsum(KK)]).astype(int)
R_TOT = int(R_OFF[-1])                          # 486
BIDX = {(3, 0): 0, (3, 1): 1, (7, 0): 2, (7, 1): 3}
TYPE_ORDER = [0, 6, 1, 5, 2, 4, 3, 7]           # small input blocks first

_nc_cache = {}


def _build_nc():
    if "nc" in _nc_cache:
        return _nc_cache["nc"]
    nc = bacc.Bacc("TRN2", target_bir_lowering=False, debug=False)
    xd = nc.dram_tensor("xd", [R_TOT, NPT], _F16, kind="ExternalInput").ap()
    wtb = nc.dram_tensor("wtb", [128, 2 * N_TYPES * 128], _F16, kind="ExternalInput").ap()
    bvec = nc.dram_tensor("bvec", [128, 4], _F32, kind="ExternalInput").ap()
    y = nc.dram_tensor("y", [2 * N_TYPES, 128, NPT], _F16, kind="ExternalOutput").ap()

    ident = mybir.ActivationFunctionType.Identity

    with tile.TileContext(nc) as tc:
        with (
            tc.tile_pool(name="const", bufs=1) as const,
            tc.tile_pool(name="xin", bufs=2) as xin_pool,
            tc.tile_pool(name="outsb", bufs=2) as out_pool,
            tc.tile_pool(name="ps", bufs=8, space="PSUM") as ps_pool,
        ):
            wtb_sb = const.tile([128, 2 * N_TYPES * 128], _F16)
            nc.sync.dma_start(wtb_sb[:], wtb[:])
            bv_sb = const.tile([128, 4], _F32)
            nc.sync.dma_start(bv_sb[:], bvec[:])

            ev = 0  # global eviction counter: 5:4 Scalar:Vector split
            for k in TYPE_ORDER:
                kk = KK[k]
                xs = xin_pool.tile([128, NPT], _F16, tag="xs", name=f"xs_{k}")
                nc.sync.dma_start(xs[0:kk, :], xd[R_OFF[k]:R_OFF[k] + kk, :])
                out_sb = out_pool.tile([128, 2 * NPT], _F16, tag="os", name=f"os_{k}")
                for h in range(2):
                    w_ap = wtb_sb[0:kk, (2 * k + h) * 128:(2 * k + h + 1) * 128]
                    off = 0
                    for ci, cw in enumerate(CHUNKS):
                        ps = ps_pool.tile([128, 512], _F32, tag="ps",
                                          name=f"ps_{k}_{h}_{ci}")
                        nc.tensor.matmul(
                            ps[:, 0:cw], w_ap, xs[0:kk, off:off + cw],
                            start=True, stop=True,
                        )
                        dst = out_sb[:, h * NPT + off:h * NPT + off + cw]
                        use_act = (ev % 9) < 5
                        ev += 1
                        if k in (3, 7):
                            j = BIDX[(k, h)]
                            if use_act:
                                nc.scalar.activation(dst, ps[:, 0:cw], ident,
                                                     bias=bv_sb[:, j:j + 1])
                            else:
                                nc.vector.tensor_scalar_add(dst, ps[:, 0:cw],
                                                            bv_sb[:, j:j + 1])
                        else:
                            if use_act:
                                nc.scalar.copy(dst, ps[:, 0:cw])
                            else:
                                nc.vector.tensor_copy(dst, ps[:, 0:cw])
                        off += cw
                    nc.gpsimd.dma_start(y[2 * k + h],
                                        out_sb[:, h * NPT:(h + 1) * NPT])

    nc.finalize()
    _nc_cache["nc"] = nc
    return nc


def _prep_weights(W, b):
    mask = (np.arange(MAX_DIM)[None, None, :] < NODE_DIMS[:, None, None])
    W_eff = np.where(mask, W, 0).astype(np.float32)  # [T, F, D]
    wtb = np.zeros((128, 2 * N_TYPES * 128), dtype=np.float32)
    for k in range(N_TYPES):
        d = int(NODE_DIMS[k])
        for h in range(2):
            c0 = (2 * k + h) * 128
            wtb[0:d, c0:c0 + 128] = W_eff[k, h * 128:(h + 1) * 128, :d].T
            if FOLD[k]:
                wtb[d, c0:c0 + 128] = b[k, h * 128:(h + 1) * 128]
    bvec = np.zeros((128, 4), dtype=np.float32)
    for (k, h), j in BIDX.items():
        bvec[:, j] = b[k, h * 128:(h + 1) * 128]
    return wtb.astype(np.float16), bvec


def _prep_x_shard(x, c):
    """fp16 type-major transposed layout: xd[R_OFF[k]+d, i] = x_k[i, d]
    where x_k[i] = x[c*100000 + 8*i + k] (node i of type k on core c),
    with a ones-row at d = dim_k for the bias-folding types."""
    xc = x[c * NODES_PER_CORE:(c + 1) * NODES_PER_CORE]
    xd = np.zeros((R_TOT, NPT), dtype=np.float16)
    for k in range(N_TYPES):
        d = int(NODE_DIMS[k])
        xk = xc[k::N_TYPES, :d]                       # [12500, d] f32
        xd[R_OFF[k]:R_OFF[k] + d, :NPT_REAL] = xk.astype(np.float16).T
        if FOLD[k]:
            xd[R_OFF[k] + d, :NPT_REAL] = 1.0
    return xd


def run(x, W, b, trace=False):
    nc = _build_nc()
    wtb, bvec = _prep_weights(W, b)
    in_maps = []
    for c in range(N_CORES):
        in_maps.append({
            "xd": _prep_x_shard(x, c),
            "wtb": wtb,
            "bvec": bvec,
        })
    res = run_bass_kernel_spmd(nc, in_maps, list(range(N_CORES)), trace=trace)
    y = np.empty((N_GRAPHS * N_TYPES, FEAT), dtype=np.float32)
    for c in range(N_CORES):
        yd = np.asarray(res.results[c]["y"]).astype(np.float32)
        yd = yd.reshape(N_TYPES, 2, 128, NPT)         # [k, h, p, i]
        yc = yd.transpose(3, 0, 1, 2).reshape(NPT, N_TYPES, FEAT)[:NPT_REAL]
        y[c * NODES_PER_CORE:(c + 1) * NODES_PER_CORE] = yc.reshape(
            NODES_PER_CORE, FEAT)
    return y, res


def kernel(**inputs):
    y, _ = run(inputs["x"], inputs["W"], inputs["b"])
    return y


if __name__ == "__main__":
    rng = np.random.default_rng(0)
    x = rng.standard_normal((N_GRAPHS * N_TYPES, MAX_DIM), dtype=np.float32)
    W = (rng.standard_normal((N_TYPES, FEAT, MAX_DIM), dtype=np.float32) * 0.05)
    b = (rng.standard_normal((N_TYPES, FEAT), dtype=np.float32) * 0.05)
    y, res = run(x, W, b)
    mask = (np.arange(MAX_DIM)[None, None, :] < NODE_DIMS[:, None, None])
    W_eff = np.where(mask, W, 0).astype(np.float32)
    idx = rng.integers(0, N_GRAPHS * N_TYPES, 256)
    exp = np.stack([W_eff[n % 8] @ x[n] + b[n % 8] for n in idx])
    act = y[idx]
    err = np.abs(act - exp).max() / (np.abs(exp).max() + 1e-30)
    print("spot-check rel err:", err)
